# revision 1
# baseline (speedup 1.0000x reference)
"""EmbeddingBag-mean (padded ragged gather + masked mean) on 8 Trainium2 cores.

Strategy (data-parallel over batch, per the sharding hint):
  - Each of the 8 cores owns B/8 = 2048 samples; the embedding table is
    replicated to every core's HBM (augmented with one zero row at index V).
  - Host prep: indices -> int32; within each core, samples are sorted by
    descending length so each block of 128 samples only needs G_b =
    max-length-in-block gather slots; padded slots point at the zero row.
  - Device kernel (per core), per block of 128 samples:
      1. G_b indirect DMA gathers (one index per partition per slot):
         g[p, l, :] = table[idx[p, l], :]
      2. one DVE tensor_reduce over slots (strided AP view [P, D, G_b])
      3. ACT Copy-with-scale by 1/max(len,1) (per-partition scalar)
      4. DMA the [128, 64] block out
  - Host un-permutes (inverse of the length sort) and concatenates the
    8 per-core outputs.

The per-block slot counts G_b depend on the input lengths, so the Bass
module is built per distinct slot schedule (cached).
"""

import numpy as np

try:
    import concourse.bacc as bacc
except ImportError:  # harness containers keep the repo at /opt/trn_rl_repo
    import sys

    sys.path.insert(0, "/opt/trn_rl_repo")
    import concourse.bacc as bacc

import concourse.bass as bass
import concourse.mybir as mybir
import concourse.tile as tile
from concourse import bass_utils

B, L, V, D = 16384, 50, 100000, 64
NCORES = 8
P = 128
BC = B // NCORES  # 2048 samples per core
NBLK = BC // P  # 16 blocks of 128 samples

_CACHE: dict = {}


def build(g_list, reps: int = 1, gbufs: int = 4):
    """Build + bacc-compile the per-core Bass module.

    g_list: per-block gather slot counts (len NBLK, each in [1, L]).
    reps > 1 wraps the block loop in tc.For_i (same outputs each
    iteration) -- used only for wall-clock slope timing in test.py.
    """
    g_list = list(g_list)
    assert len(g_list) == NBLK and all(1 <= g <= L for g in g_list)
    W = sum(g_list)
    offs = np.cumsum([0] + g_list).tolist()
    g_max = max(g_list)

    nc = bacc.Bacc("TRN2", target_bir_lowering=False, debug=False)
    table = nc.dram_tensor("table", [V + 1, D], mybir.dt.float32, kind="ExternalInput")
    idx = nc.dram_tensor("idx", [P, W], mybir.dt.int32, kind="ExternalInput")
    inv_len = nc.dram_tensor("inv_len", [P, NBLK], mybir.dt.float32, kind="ExternalInput")
    out = nc.dram_tensor("out", [NBLK, P, D], mybir.dt.float32, kind="ExternalOutput")

    with tile.TileContext(nc) as tc:
        with (
            tc.tile_pool(name="const", bufs=1) as cpool,
            tc.tile_pool(name="gather", bufs=gbufs) as gpool,
            tc.tile_pool(name="res", bufs=4) as rpool,
        ):
            idx_sb = cpool.tile([P, W], mybir.dt.int32)
            nc.sync.dma_start(idx_sb[:], idx.ap())
            invl_sb = cpool.tile([P, NBLK], mybir.dt.float32)
            nc.sync.dma_start(invl_sb[:], inv_len.ap())

            def body():
                for b in range(NBLK):
                    gb = g_list[b]
                    g = gpool.tile([P, g_max, D], mybir.dt.float32, tag="g")
                    gflat = g[:].rearrange("p l d -> p (l d)")
                    for l in range(gb):
                        nc.gpsimd.indirect_dma_start(
                            out=gflat[:, l * D : (l + 1) * D],
                            out_offset=None,
                            in_=table.ap(),
                            in_offset=bass.IndirectOffsetOnAxis(
                                ap=idx_sb[:, offs[b] + l : offs[b] + l + 1], axis=0
                            ),
                        )
                    red = rpool.tile([P, D], mybir.dt.float32, tag="red")
                    nc.vector.tensor_reduce(
                        out=red[:],
                        in_=g[:, :gb, :].rearrange("p l d -> p d l"),
                        axis=mybir.AxisListType.X,
                        op=mybir.AluOpType.add,
                    )
                    o = rpool.tile([P, D], mybir.dt.float32, tag="o")
                    nc.scalar.activation(
                        o[:],
                        red[:],
                        mybir.ActivationFunctionType.Copy,
                        scale=invl_sb[:, b : b + 1],
                    )
                    nc.sync.dma_start(out.ap()[b], o[:])

            if reps == 1:
                body()
            else:
                with tc.For_i(0, reps, 1):
                    body()

    nc.compile()
    return nc


def preprocess(table, indices, lengths):
    """Host prep. Returns (in_maps, g_list, perms) where perms[c] maps
    device row order (sorted) back to original sample order."""
    table = np.ascontiguousarray(np.asarray(table, dtype=np.float32))
    table_aug = np.concatenate([table, np.zeros((1, D), np.float32)], axis=0)

    idx32 = np.asarray(indices).astype(np.int32)  # [B, L]
    lens = np.asarray(lengths).astype(np.int64)  # [B]
    valid = np.arange(L, dtype=np.int64)[None, :] < lens[:, None]
    idx32 = np.where(valid, idx32, np.int32(V))
    inv_len = (1.0 / np.maximum(lens, 1)).astype(np.float32)  # [B]

    # Sort each core's samples by descending length; block b then only
    # needs G_b = lens_sorted[128*b] gather slots. g_list must be shared
    # across cores (one compiled module), so take the per-block max.
    perms, g_lists = [], []
    for c in range(NCORES):
        s = slice(c * BC, (c + 1) * BC)
        perm = np.argsort(-lens[s], kind="stable")
        perms.append(perm)
        ls = lens[s][perm]
        g_lists.append(np.maximum(ls[::P][:NBLK], 1))
    g_list = np.maximum.reduce(g_lists).astype(int).tolist()
    W = int(np.sum(g_list))
    offs = np.cumsum([0] + g_list)

    in_maps = []
    for c in range(NCORES):
        s = slice(c * BC, (c + 1) * BC)
        idx_c = idx32[s][perms[c]]  # [BC, L] sorted by desc length
        invl_c = inv_len[s][perms[c]]
        idx_dev = np.full((P, W), V, np.int32)
        for b in range(NBLK):
            gb = g_list[b]
            blk = idx_c[b * P : (b + 1) * P, :gb]  # [P, gb]
            idx_dev[:, offs[b] : offs[b] + gb] = blk
        invl_dev = invl_c.reshape(NBLK, P).T  # [P, NBLK]
        in_maps.append(
            {
                "table": table_aug,
                "idx": np.ascontiguousarray(idx_dev),
                "inv_len": np.ascontiguousarray(invl_dev),
            }
        )
    return in_maps, g_list, perms


def kernel(table, indices, lengths):
    in_maps, g_list, perms = preprocess(table, indices, lengths)
    key = tuple(g_list)
    nc = _CACHE.get(key)
    if nc is None:
        nc = _CACHE[key] = build(g_list)
    res = bass_utils.run_bass_kernel_spmd(nc, in_maps, core_ids=list(range(NCORES)))
    full = np.empty((B, D), np.float32)
    for c in range(NCORES):
        rows = res.results[c]["out"].reshape(BC, D)
        full[c * BC : (c + 1) * BC][perms[c]] = rows
    return full

